# revision 36
# baseline (speedup 1.0000x reference)
"""GQA attention layer (B=2, S=2048, D=4096, 32 Q heads / 8 KV heads, RoPE,
causal) on 8 Trainium2 NeuronCores, tensor-parallel over heads.

Each core owns 4 Q heads + 1 KV head: it computes its Q/K/V projections,
RoPE, causal attention, and a partial output projection (rank-512 slice of
the wo contraction).  The host sums the 8 partial outputs.

v2 design (vs the fp32r/DRAM-scratch baseline):
  * all matmul operands are bf16 (rel-err budget 2e-2; measured ~6e-3),
    halving HBM traffic and SBUF footprint and enabling FWL weight loads
  * q/k/v stay RESIDENT in SBUF between the projection and attention
    phases -- no DRAM scratch roundtrip, no DMA wait at the transition
  * causal diagonal blocks restrict the matmul moving range instead of
    computing fully-masked columns; a single [128,128] triangle mask is
    added on the sub-diagonal chunk
  * softmax denominators via DVE accumulation of exp chunks + one
    ones-row matmul per (head, q-block) instead of a 512-cycle PE
    row-sum matmul per k-chunk
  * attention inner loop is software-pipelined (scores of chunk c+1
    issue before AV of chunk c) and the previous q-block's output
    projection matmuls are interleaved into the stream so the PE never
    waits on the scalar engine's exp
"""

import os
import sys
import types
from contextlib import ExitStack

import numpy as np
import ml_dtypes

import concourse.bass as bass
import concourse.tile as tile
from concourse import bacc
from concourse import mybir
from concourse import bass_utils
from concourse.bass_utils import run_bass_kernel_spmd

# ---------------------------------------------------------------------------
# Optional NTFF profiling support under axon. The trimmed image's `antenv`
# lacks `axon_hooks`, so run_bass_kernel_spmd(trace=True) would silently skip
# tracing; register the hook ourselves. Harmless when unavailable.
try:
    import antenv  # noqa: F401
    from trn_agent_boot.trn_boot import _ntff_profile_via_ctypes

    if "antenv.axon_hooks" not in sys.modules:
        _hooks_mod = types.ModuleType("antenv.axon_hooks")
        _hook = _ntff_profile_via_ctypes("/opt/axon/libaxon_pjrt.so")
        _hooks_mod.get_axon_ntff_profile_hook = lambda: _hook
        _hooks_mod.set_axon_ntff_profile_hook = lambda h: None
        sys.modules["antenv.axon_hooks"] = _hooks_mod
    bass_utils.upload_artifacts = lambda tmpdir: "local://skipped"
except Exception:
    pass

F32 = mybir.dt.float32
F32R = mybir.dt.float32r
BF16 = mybir.dt.bfloat16
EXP = mybir.ActivationFunctionType.Exp
NPBF16 = ml_dtypes.bfloat16

B, S, D = 2, 2048, 4096
NH, NKV, HD = 32, 8, 128
T = B * S                       # 4096 tokens total
N_CORES = 8
QH = NH // N_CORES              # 4 local q heads
FL = QH * HD                    # 512 local q features
SCALE = 1.0 / float(np.sqrt(HD))
NEG = -1.0e30

NW = 512                        # token-group width in the QKV projection
QB = 512                        # q-block width in attention
DKD = D // 128                  # 32 contraction chunks for projections
NG = T // NW                    # 8 token groups


def _build_program():
    nc = bacc.Bacc("TRN2", target_bir_lowering=False, debug=False,
                   num_devices=N_CORES)

    xT = nc.dram_tensor("xT", [D, T], BF16, kind="ExternalInput").ap()
    wqT = nc.dram_tensor("wqT", [D, FL], BF16, kind="ExternalInput").ap()
    wkT = nc.dram_tensor("wkT", [D, HD], BF16, kind="ExternalInput").ap()
    wvT = nc.dram_tensor("wvT", [D, HD], BF16, kind="ExternalInput").ap()
    woT = nc.dram_tensor("woT", [FL, D], BF16, kind="ExternalInput").ap()
    # RoPE constants, pre-assembled for the rotate-half formulation on the
    # even/odd-split feature layout: ropc = [cos; cos], rops = [-sin; sin].
    ropc = nc.dram_tensor("ropc", [HD, S], BF16, kind="ExternalInput").ap()
    rops = nc.dram_tensor("rops", [HD, S], BF16, kind="ExternalInput").ap()
    idin = nc.dram_tensor("idin", [128, 128], BF16, kind="ExternalInput").ap()
    onesin = nc.dram_tensor("onesin", [128, 1], F32R, kind="ExternalInput").ap()
    onesbin = nc.dram_tensor("onesbin", [128, 1], BF16, kind="ExternalInput").ap()
    # Sub-diagonal triangle mask: trimask[r, c] = 0 if c >= r else -1e30
    maskin = nc.dram_tensor("maskin", [128, 128], BF16, kind="ExternalInput").ap()
    y = nc.dram_tensor("y", [T, D], BF16, kind="ExternalOutput").ap()

    with tile.TileContext(nc) as tc, ExitStack() as ctx:
        # ------------------------------------------------------------------
        # Whole-program resident tiles: weights, constants, q/k/v activations
        # ------------------------------------------------------------------
        const = ctx.enter_context(tc.tile_pool(name="const", bufs=1))
        ident = const.tile([128, 128], BF16)
        ones_t = const.tile([128, 1], F32R)
        ones_b = const.tile([128, 1], BF16)
        trimask = const.tile([128, 128], BF16)
        cos_s = const.tile([HD, S], BF16)
        sin_s = const.tile([HD, S], BF16)

        wpool = ctx.enter_context(tc.tile_pool(name="weights", bufs=1))
        # Resident weights, packed k-chunk-major: [128, DKD * width].
        # Chunk-interleaved DMAs so the first matmul group can start after
        # ~0.5 MiB instead of waiting for all 10 MiB of weights.
        wq_sb = wpool.tile([128, DKD * FL], BF16, tag="wq")
        wk_sb = wpool.tile([128, DKD * HD], BF16, tag="wk")
        wv_sb = wpool.tile([128, DKD * HD], BF16, tag="wv")
        wo_sb = wpool.tile([128, QH * D], BF16, tag="wo")
        def dma_w_chunk(j, k4=None):
            k4 = k4 if k4 is not None else slice(4 * j, 4 * j + 4)
            nc.sync.dma_start(
                wq_sb[:].rearrange("p (k f) -> p k f", k=DKD)[:, k4, :],
                wqT.rearrange("(k p) f -> p k f", p=128)[:, k4, :])
            nc.sync.dma_start(
                wk_sb[:].rearrange("p (k f) -> p k f", k=DKD)[:, k4, :],
                wkT.rearrange("(k p) f -> p k f", p=128)[:, k4, :])
            nc.sync.dma_start(
                wv_sb[:].rearrange("p (k f) -> p k f", k=DKD)[:, k4, :],
                wvT.rearrange("(k p) f -> p k f", p=128)[:, k4, :])

        dma_w_chunk(0, slice(0, 1))
        dma_w_chunk(0, slice(1, 4))

        vtpool = ctx.enter_context(tc.tile_pool(name="vtrans", bufs=2))
        ptpool = ctx.enter_context(tc.tile_pool(name="ptiles", bufs=6))
        resid = ctx.enter_context(tc.tile_pool(name="resid", bufs=1))
        q_res = [[resid.tile([128, S], BF16, tag=f"q{b}_{m}", name=f"q{b}_{m}")
                  for m in range(QH)] for b in range(B)]
        k_res = [resid.tile([128, S], BF16, tag=f"k{b}", name=f"k{b}")
                 for b in range(B)]
        v_res = [resid.tile([128, S], BF16, tag=f"v{b}", name=f"v{b}")
                 for b in range(B)]

        V_bs = []
        # ------------------------------------------------------------------
        # Phase 1: QKV projections + RoPE -> resident SBUF (feature-major)
        # ------------------------------------------------------------------
        with tc.tile_pool(name="xin", bufs=6) as xpool, \
             tc.tile_pool(name="qkvstage", bufs=2) as stage, \
             tc.tile_pool(name="ropetmp", bufs=2) as rtmp, \
             tc.tile_pool(name="vtps", bufs=2, space="PSUM") as vtpsum, \
             tc.tile_pool(name="qkvps", bufs=1, space="PSUM") as qkvps:

            def rope_emit(src, dst, pos0):
                """dst = RoPE(src) on the even/odd-split feature layout
                (partitions 0..63 even pair components, 64..127 odd):
                dst = src * [c;c] + swap_halves(src) * [-s;s]."""
                c = cos_s[:, pos0:pos0 + NW]
                s = sin_s[:, pos0:pos0 + NW]
                xsw = rtmp.tile([128, NW], F32, tag="xsw")
                nc.sync.dma_start(xsw[0:64, :], src[64:128, :])
                nc.sync.dma_start(xsw[64:128, :], src[0:64, :])
                t1 = rtmp.tile([128, NW], F32, tag="t1")
                nc.vector.tensor_mul(t1[:], xsw[:], s)
                t2 = rtmp.tile([128, NW], F32, tag="t2")
                nc.vector.tensor_mul(t2[:], src[:], c)
                nc.vector.tensor_add(dst, t2[:], t1[:])

            for n in range(NG):
                b = n * NW // S
                pos0 = (n * NW) % S
                if n == 1:
                    nc.sync.dma_start(ident[:], idin)
                    nc.sync.dma_start(ones_t[:], onesin)
                    nc.sync.dma_start(ones_b[:], onesbin)
                    nc.sync.dma_start(trimask[:], maskin)
                if 1 <= n <= 4:
                    # output-projection weights: prefetch in 1 MiB chunks
                    # during groups 1-4 so they never stall the x stream
                    fd = wo_sb[:].rearrange("p (f d) -> p f d", f=QH)
                    sd = woT.rearrange("(f p) d -> p f d", p=128)
                    nc.sync.dma_start(fd[:, n - 1:n, :], sd[:, n - 1:n, :])
                qps = [qkvps.tile([128, NW], F32, tag=f"qps{m}", name=f"qps{m}")
                       for m in range(QH)]
                kps = qkvps.tile([128, NW], F32, tag="kps")
                vps = qkvps.tile([128, NW], F32, tag="vps")
                for j in range(DKD // 4):
                    xt4 = xpool.tile([128, 4 * NW], BF16)
                    nc.sync.dma_start(
                        xt4[:].rearrange("p (k t) -> p k t", k=4),
                        xT.rearrange("(k p) t -> p k t", p=128)[
                            :, 4 * j:4 * j + 4, n * NW:(n + 1) * NW])
                    if n == 0 and j < 7:
                        dma_w_chunk(j + 1)
                    if n == 0 and j == 7:
                        # needed by this group's RoPE, after the last x tile
                        nc.sync.dma_start(cos_s[:], ropc)
                        nc.sync.dma_start(sin_s[:], rops)
                    for kk in range(4):
                        k = 4 * j + kk
                        xt = xt4[:, kk * NW:(kk + 1) * NW]
                        st = (k == 0)
                        sp = (k == DKD - 1)
                        for m in range(QH):
                            nc.tensor.matmul(
                                qps[m][:],
                                wq_sb[:, k * FL + m * 128:k * FL + (m + 1) * 128],
                                xt, start=st, stop=sp)
                        nc.tensor.matmul(
                            kps[:], wk_sb[:, k * HD:(k + 1) * HD], xt,
                            start=st, stop=sp)
                        nc.tensor.matmul(
                            vps[:], wv_sb[:, k * HD:(k + 1) * HD], xt,
                            start=st, stop=sp)
                # Evict all 6 PSUM accumulators first (frees banks for the
                # next group ASAP), alternating ACT/DVE; then RoPE math.
                qc = []
                for m in range(QH):
                    t = stage.tile([128, NW], F32, tag=f"qc{m}", name=f"qc{m}")
                    if m % 2 == 0:
                        nc.scalar.copy(t[:], qps[m][:])
                    else:
                        nc.vector.tensor_copy(t[:], qps[m][:])
                    qc.append(t)
                kc = stage.tile([128, NW], F32, tag="kc")
                nc.scalar.copy(kc[:], kps[:])
                # v needs no RoPE: cast straight into the resident tile.
                nc.vector.tensor_copy(v_res[b][:, pos0:pos0 + NW], vps[:])
                for m in range(QH):
                    rope_emit(qc[m], q_res[b][m][:, pos0:pos0 + NW], pos0)
                rope_emit(kc, k_res[b][:, pos0:pos0 + NW], pos0)
                if n % 4 == 3:
                    # batch b's V is complete: build the token-major copy
                    # V_bs[b][:, kc*128:+128] = v_res[b][:, chunk].T while
                    # the next group's projections keep the PE warm
                    Vb = vtpool.tile([128, S], BF16, tag="V_b", name="V_b")
                    for kc2 in range(S // 128):
                        vt_ps = vtpsum.tile([128, 128], BF16, tag="vtp",
                                            name="vtp")
                        nc.tensor.transpose(
                            vt_ps[:],
                            v_res[b][:, kc2 * 128:(kc2 + 1) * 128], ident[:])
                        nc.scalar.copy(
                            Vb[:, kc2 * 128:(kc2 + 1) * 128], vt_ps[:])
                    V_bs.append(Vb)

        # ------------------------------------------------------------------
        # Phase 2: attention + output projection
        # ------------------------------------------------------------------
        with tc.tile_pool(name="pacc", bufs=2) as papool, \
             tc.tile_pool(name="attn", bufs=2) as atpool, \
             tc.tile_pool(name="smax", bufs=2) as smpool, \
             tc.tile_pool(name="ystage", bufs=2) as ypool, \
             tc.tile_pool(name="sps", bufs=4, space="PSUM") as spsum, \
             tc.tile_pool(name="avps", bufs=2, space="PSUM") as avpsum, \
             tc.tile_pool(name="yps", bufs=2, space="PSUM") as ypsum:

            def wo_gen(att_prev, b_prev, q0_prev):
                """Output projection for a finished q block, as a generator
                that yields once per PE matmul so the caller can interleave
                them into the attention stream."""
                for tcx in range(QB // 128):
                    tg0 = b_prev * S + q0_prev + tcx * 128
                    for half in range(2):
                        ysb = ypool.tile([128, D // 2], BF16, tag="ysb",
                                         name="ysb")
                        for dgh in range(4):
                            dg = half * 4 + dgh
                            yp = ypsum.tile([128, NW], F32, tag="yp", name="yp")
                            for f in range(QH):
                                nc.tensor.matmul(
                                    yp[:],
                                    att_prev[f][:, tcx * 128:(tcx + 1) * 128],
                                    wo_sb[:, f * D + dg * NW:f * D + (dg + 1) * NW],
                                    start=(f == 0), stop=(f == QH - 1))
                                yield
                            if dgh % 2 == 0:
                                nc.scalar.copy(
                                    ysb[:, dgh * NW:(dgh + 1) * NW], yp[:])
                            else:
                                nc.vector.tensor_copy(
                                    ysb[:, dgh * NW:(dgh + 1) * NW], yp[:])
                        nc.sync.dma_start(
                            y[tg0:tg0 + 128,
                              half * (D // 2):(half + 1) * (D // 2)], ysb[:])

            def drain(gen, k):
                if gen is None:
                    return
                for _ in range(k):
                    try:
                        next(gen)
                    except StopIteration:
                        return

            pending = None
            units = [(b, qb, h) for b in range(B)
                     for qb in range(S // QB) for h in range(QH)]

            def u_nkt(u):
                return (u[1] + 1) * (QB // 128)

            def chunk_geom(u, c):
                vv = c - (u_nkt(u) - 4)
                if vv >= 0:
                    return vv * 128, (4 - vv) * 128  # qoff, width
                return 0, QB

            flat = [(i, c) for i, u in enumerate(units)
                    for c in range(u_nkt(u))]
            score_tiles = {}

            def emit_score(i, c):
                ub, uqb, uh = units[i]
                nkt_u = u_nkt(units[i])
                qoff, w = chunk_geom(units[i], c)
                diag = c >= nkt_u - 4
                t = spsum.tile([128, QB], F32, tag="stp", name="stp")
                nc.tensor.matmul(
                    t[:, 0:w], k_res[ub][:, c * 128:(c + 1) * 128],
                    q_res[ub][uh][:, uqb * QB + qoff:uqb * QB + qoff + w],
                    start=True, stop=not diag)
                if diag:
                    # causal triangle applied on the PE (128-cycle
                    # accumulating ident @ trimask) -- keeps the DVE out
                    # of the scores -> exp dependency chain
                    nc.tensor.matmul(
                        t[:, 0:128], ident[:], trimask[:],
                        start=False, stop=True)
                score_tiles[(i, c)] = t

            def emit_score_pos(p):
                if p < len(flat):
                    i, c = flat[p]
                    if (i, c) not in score_tiles:
                        emit_score(i, c)

            # Normalization of head h is deferred into head h+1's chunk
            # stream: the rowsum matmul and its 4-engine chain (ACT copy,
            # gpsimd broadcast, DVE reciprocal+mul) run while h+1's
            # scores/AV keep the PE fed, instead of blocking it.
            def make_norm(avp, pacc, att_t):
                def norm(smp_u=smp_u):
                    smp = smp_u[0]
                    s1 = smpool.tile([1, QB], F32, tag="s1", name="s1")
                    nc.scalar.copy(s1[:], smp[0:1, :])
                    s_bc = smpool.tile([128, QB], F32, tag="s_bc")
                    nc.gpsimd.partition_broadcast(s_bc[:], s1[:])
                    r_bc = smpool.tile([128, QB], F32, tag="r_bc")
                    nc.vector.reciprocal_approx_fast(r_bc[:], s_bc[:])
                    nc.vector.tensor_mul(att_t[:], avp[:], r_bc[:])
                return norm

            pos = 0
            deferred = None
            att = None
            emit_score_pos(0)
            emit_score_pos(1)
            emit_score_pos(2)
            emit_score_pos(3)
            for i, u in enumerate(units):
                b, qb, h = u
                nkt = u_nkt(u)
                V_b = V_bs[b]
                fresh = (h == 0)
                if h == 0:
                    att = [atpool.tile([128, QB], BF16, tag=f"att{hh}",
                                       name=f"att{hh}") for hh in range(QH)]
                wo_per_chunk = -(-(QB // 128 * 8) // nkt)   # ceil
                avp = avpsum.tile([128, QB], F32, tag="avp", name="avp")
                pacc = papool.tile([128, QB], F32R, tag="pacc", name="pacc")
                smp_u = [None]

                for c in range(nkt):
                    if (i, c) not in score_tiles:
                        emit_score(i, c)
                    stp_t = score_tiles.pop((i, c))
                    qoff, w = chunk_geom(u, c)
                    pt_t = ptpool.tile([128, QB], BF16, tag="pt", name="pt")
                    nc.scalar.activation(
                        pt_t[:, 0:w], stp_t[:, 0:w], EXP, scale=SCALE)
                    nc.tensor.matmul(
                        avp[:, qoff:qoff + w],
                        V_b[:, c * 128:(c + 1) * 128], pt_t[:, 0:w],
                        start=(c == 0), stop=(c == nkt - 1))
                    emit_score_pos(pos + 4)
                    if c == 0:
                        nc.vector.tensor_copy(pacc[:], pt_t[:])
                    elif c < nkt - 2:
                        nc.vector.tensor_add(
                            pacc[:, qoff:qoff + w],
                            pacc[:, qoff:qoff + w], pt_t[:, 0:w])
                    if c == nkt - 2:
                        # denominators: pacc (chunks 0..nkt-3) + direct
                        # row-sums of the last two pt chunks -- smp is
                        # complete by unit end and the DVE chain is two
                        # adds shorter
                        smp_u[0] = ypsum.tile([128, QB], F32, tag="yp",
                                              name="yp")
                        nc.tensor.matmul(
                            smp_u[0][0:1, :], ones_t[:], pacc[:],
                            start=True, stop=False)
                        nc.tensor.matmul(
                            smp_u[0][0:1, qoff:qoff + w], ones_b[:],
                            pt_t[:, 0:w], start=False, stop=False)
                    elif c == nkt - 1:
                        nc.tensor.matmul(
                            smp_u[0][0:1, qoff:qoff + w], ones_b[:],
                            pt_t[:, 0:w], start=False, stop=True)
                    if c == 1 and deferred is not None:
                        deferred()
                        deferred = None
                    if not (fresh and c < 2):
                        drain(pending, wo_per_chunk)
                    pos += 1
                if deferred is not None:   # nkt < 2 can't happen, but be safe
                    deferred()
                    deferred = None
                norm = make_norm(avp, pacc, att[h])
                if h < QH - 1:
                    deferred = norm
                else:
                    norm()
                    drain(pending, 10 ** 9)
                    pending = wo_gen(att, b, qb * QB)
            if deferred is not None:
                deferred()
            drain(pending, 10 ** 9)
    nc.compile()
    return nc


_program = None


def _get_program():
    global _program
    if _program is None:
        _program = _build_program()
    return _program


def kernel(**inputs) -> np.ndarray:
    x = np.asarray(inputs["x"], dtype=np.float32)
    wq = np.asarray(inputs["wq"], dtype=np.float32)
    wk = np.asarray(inputs["wk"], dtype=np.float32)
    wv = np.asarray(inputs["wv"], dtype=np.float32)
    wo = np.asarray(inputs["wo"], dtype=np.float32)
    cos = np.asarray(inputs["freqs_cos"], dtype=np.float32)
    sin = np.asarray(inputs["freqs_sin"], dtype=np.float32)
    start_pos = int(np.asarray(inputs.get("start_pos", 0)))
    assert start_pos == 0, "kernel specialized for start_pos == 0"

    # Even/odd RoPE pair split within each head's 128 features.
    perm = np.concatenate([np.arange(0, HD, 2), np.arange(1, HD, 2)])

    xT = np.ascontiguousarray(x.reshape(T, D).T.astype(NPBF16))
    cosT = cos.T                                   # [64, S]
    sinT = sin.T
    ropc = np.ascontiguousarray(
        np.concatenate([cosT, cosT], axis=0).astype(NPBF16))
    rops = np.ascontiguousarray(
        np.concatenate([-sinT, sinT], axis=0).astype(NPBF16))
    rr, cc = np.meshgrid(np.arange(128), np.arange(128), indexing="ij")
    maskin = np.where(cc >= rr, 0.0, NEG).astype(np.float32).astype(NPBF16)

    in_maps = []
    for c in range(N_CORES):
        wq_c = wq[c * FL:(c + 1) * FL].reshape(QH, HD, D)[:, perm, :].reshape(FL, D)
        wk_c = wk[c * HD:(c + 1) * HD][perm, :]
        wv_c = wv[c * HD:(c + 1) * HD]
        wo_c = wo[:, c * FL:(c + 1) * FL]
        in_maps.append({
            "xT": xT,
            "idin": np.eye(128, dtype=np.float32).astype(NPBF16),
            "wqT": np.ascontiguousarray(wq_c.T.astype(NPBF16)),
            "wkT": np.ascontiguousarray(wk_c.T.astype(NPBF16)),
            "wvT": np.ascontiguousarray(wv_c.T.astype(NPBF16)),
            "woT": np.ascontiguousarray(wo_c.T.astype(NPBF16)),
            "ropc": ropc,
            "rops": rops,
            "onesin": np.ones((128, 1), dtype=np.float32),
            "onesbin": np.ones((128, 1), dtype=np.float32).astype(NPBF16),
            "maskin": maskin,
        })

    nc = _get_program()
    trace = bool(int(os.environ.get("GQA_TRACE", "0")))
    kwargs = {}
    if trace:
        tmpdir = os.environ.get("GQA_TRACE_DIR") or None
        kwargs = dict(trace=True, tmpdir=tmpdir, trace_cores=[0])
    res = run_bass_kernel_spmd(nc, in_maps, list(range(N_CORES)), **kwargs)
    kernel.last_results = res

    acc = np.zeros((T, D), dtype=np.float32)
    for c in range(N_CORES):
        acc += np.asarray(res.results[c]["y"]).astype(np.float32)
    return acc.reshape(B, S, D)
